# revision 4
# baseline (speedup 1.0000x reference)
"""Trainium2 Bass kernel for nn_ConeGeometryNet.

Math: for each point p=(px,py) and each 2x2 cone matrix `dirs` (rows are the
two edge directions), the reference computes
  - containment: lambda = solve(dirs^T, p) >= 0 (both coords)
  - distance:    min over the two edge lines of perpendicular distance |cross(unit_edge, p)|

Both reduce to two linear functionals of p per matrix.  With
  s0 = d*px - c*py  (= lambda0 * det),   s1 = a*py - b*px  (= lambda1 * det),
  n0 = |(a,b)|, n1 = |(c,d)|, sgn = sign(det):
  u = sgn*s1/n0,  v = sgn*s0/n1
then
  inside = (lambda0>=0 & lambda1>=0) = (min(u,v) >= 0)
  dist   = min(|u|, |v|)
  inner  = inside*dist = relu(min(u,v))      (when inside, dist == min(u,v))
  outer  = dist - inner
So the per-point work is a (2 -> 2*500) matmul with host-precomputed
coefficients plus a handful of elementwise ops, and the kernel is dominated
by ~324 MB of output DMA writes (memory-bound, as expected).

Sharding: data-parallel over points, 8 cores x 6250 points; frame/window
coefficients replicated.  Outputs concatenated on axis 0 on the host.
"""

import os
import sys

import numpy as np

for _p in ("/opt/trn_rl_repo",):
    if os.path.isdir(_p) and _p not in sys.path:
        sys.path.insert(0, _p)

N, C, H, W, D = 50000, 20, 5, 4, 2
NCORES = 8
NS = N // NCORES          # 6250 points per core
TP = 125                  # points per tile (partition dim)
NT = NS // TP             # 50 tiles per core
NF = C * H                # 100 frame cones
NW = C * H * W            # 400 window cones
NM = NF + NW              # 500 cones total
I32_COLS = NM + NF + C    # inside(500) | house(100) | containment(20)
F32_COLS = 2 * NM         # inner(500) | outer(500)

_RUNNER = None


def _coeffs(frame, windows):
    """(2, 2*NM) f32 coefficient matrix: cols [u_0..u_499 | v_0..v_499]."""
    mats = np.concatenate(
        [np.asarray(frame, np.float64).reshape(-1, 2, 2),
         np.asarray(windows, np.float64).reshape(-1, 2, 2)], axis=0)  # (NM,2,2)
    a, b = mats[:, 0, 0], mats[:, 0, 1]
    c_, d = mats[:, 1, 0], mats[:, 1, 1]
    det = a * d - b * c_
    sgn = np.where(det >= 0.0, 1.0, -1.0)
    n0 = np.hypot(a, b)
    n1 = np.hypot(c_, d)
    K = np.empty((2, 2 * NM), np.float64)
    K[0, :NM] = -sgn * b / n0      # u = sgn*(a*py - b*px)/n0
    K[1, :NM] = sgn * a / n0
    K[0, NM:] = sgn * d / n1       # v = sgn*(d*px - c*py)/n1
    K[1, NM:] = -sgn * c_ / n1
    return np.ascontiguousarray(K.astype(np.float32))


def _build_nc():
    import concourse.bacc as bacc
    import concourse.mybir as mybir
    from concourse import tile

    dt = mybir.dt
    Alu = mybir.AluOpType
    Act = mybir.ActivationFunctionType
    Axis = mybir.AxisListType

    nc = bacc.Bacc(None, target_bir_lowering=False)
    pts = nc.declare_dram_parameter("points", [NS, D], dt.float32, isOutput=False)
    cof = nc.declare_dram_parameter("coeffs", [D, 2 * NM], dt.float32, isOutput=False)
    o_f32 = nc.declare_dram_parameter("out_f32", [NS, F32_COLS], dt.float32, isOutput=True)
    o_i32 = nc.declare_dram_parameter("out_i32", [NS, I32_COLS], dt.int32, isOutput=True)

    with tile.TileContext(nc) as tc:
        with (
            tc.tile_pool(name="const", bufs=1) as cpool,
            tc.tile_pool(name="work", bufs=3) as wpool,
            tc.tile_pool(name="pu", bufs=3, space="PSUM") as pu,
            tc.tile_pool(name="pv", bufs=3, space="PSUM") as pv,
        ):
            # Transposed points (2, NS) so each tile's lhsT is a free-dim slice.
            PT = cpool.tile([D, NS], dt.float32)
            CHUNK = 1250
            for c0 in range(0, NS, CHUNK):
                nc.sync.dma_start(
                    out=PT[:, c0:c0 + CHUNK],
                    in_=pts[c0:c0 + CHUNK, :].rearrange("n d -> d n"),
                )
            K = cpool.tile([D, 2 * NM], dt.float32)
            nc.sync.dma_start(out=K[:], in_=cof[:, :])

            for t in range(NT):
                lhsT = PT[:, t * TP:(t + 1) * TP]
                U = pu.tile([TP, NM], dt.float32)
                V = pv.tile([TP, NM], dt.float32)
                nc.tensor.matmul(U[:], lhsT, K[:, 0:NM], start=True, stop=True)
                nc.tensor.matmul(V[:], lhsT, K[:, NM:2 * NM], start=True, stop=True)

                mn = wpool.tile([TP, NM], dt.float32)
                vs = wpool.tile([TP, NM], dt.float32)
                au = wpool.tile([TP, NM], dt.float32)
                av = wpool.tile([TP, NM], dt.float32)
                dd = wpool.tile([TP, NM], dt.float32)
                fout = wpool.tile([TP, F32_COLS], dt.float32)
                iout = wpool.tile([TP, I32_COLS], dt.int32)
                maxw = wpool.tile([TP, NF], dt.int32)

                # tensor_tensor may read at most one PSUM operand -> stage V in SBUF
                nc.vector.tensor_copy(vs[:], V[:])
                nc.vector.tensor_tensor(mn[:], U[:], vs[:], op=Alu.min)
                nc.scalar.activation(au[:], U[:], Act.Abs)
                nc.scalar.activation(av[:], vs[:], Act.Abs)
                # inner = relu(min(u,v))
                nc.scalar.activation(fout[:, 0:NM], mn[:], Act.Relu)
                # dist = min(|u|, |v|);  outer = dist - inner  (Pool: no min, DVE does it)
                nc.vector.tensor_tensor(dd[:], au[:], av[:], op=Alu.min)
                nc.gpsimd.tensor_tensor(fout[:, NM:2 * NM], dd[:], fout[:, 0:NM], op=Alu.subtract)
                # inside = (min(u,v) >= 0) as int32
                nc.gpsimd.tensor_scalar(iout[:, 0:NM], mn[:], 0.0, None, op0=Alu.is_ge)
                # house = insideF & !max_w(insideW)  ==  insideF > max_w(insideW)
                nc.vector.tensor_reduce(
                    maxw[:], iout[:, NF:NM].rearrange("p (g w) -> p g w", w=W),
                    axis=Axis.X, op=Alu.max)
                nc.vector.tensor_tensor(iout[:, NM:NM + NF], iout[:, 0:NF], maxw[:], op=Alu.is_gt)
                # containment = max_h(house)
                nc.vector.tensor_reduce(
                    iout[:, NM + NF:I32_COLS],
                    iout[:, NM:NM + NF].rearrange("p (c h) -> p c h", h=H),
                    axis=Axis.X, op=Alu.max)

                nc.sync.dma_start(out=o_f32[t * TP:(t + 1) * TP, :], in_=fout[:])
                nc.sync.dma_start(out=o_i32[t * TP:(t + 1) * TP, :], in_=iout[:])

    nc.finalize()
    return nc


class _Runner:
    def __init__(self):
        self.nc = _build_nc()

    def run(self, pts, Kc):
        from concourse.bass_utils import run_bass_kernel_spmd

        in_maps = [
            {"points": pts[i * NS:(i + 1) * NS], "coeffs": Kc}
            for i in range(NCORES)
        ]
        return run_bass_kernel_spmd(self.nc, in_maps, list(range(NCORES))).results


def _get_runner():
    global _RUNNER
    if _RUNNER is None:
        _RUNNER = _Runner()
    return _RUNNER


def kernel(points, frame, windows):
    pts = np.ascontiguousarray(np.asarray(points, dtype=np.float32))
    assert pts.shape == (N, D), pts.shape
    Kc = _coeffs(frame, windows)

    rr = _get_runner().run(pts, Kc)
    f = np.concatenate([r["out_f32"] for r in rr], axis=0)
    ii = np.concatenate([r["out_i32"] for r in rr], axis=0)

    return dict(
        embeddings=pts,
        crisp_frame_containment=np.ascontiguousarray(ii[:, 0:NF]).reshape(N, C, H),
        crisp_window_containment=np.ascontiguousarray(ii[:, NF:NM]).reshape(N, C, H, W),
        crisp_house_containment=np.ascontiguousarray(ii[:, NM:NM + NF]).reshape(N, C, H),
        crisp_containment=np.ascontiguousarray(ii[:, NM + NF:I32_COLS]),
        inner_frame_distance=np.ascontiguousarray(f[:, 0:NF]).reshape(N, C, H),
        outer_frame_distance=np.ascontiguousarray(f[:, NM:NM + NF]).reshape(N, C, H),
        inner_window_distances=np.ascontiguousarray(f[:, NF:NM]).reshape(N, C, H, W),
        outer_window_distances=np.ascontiguousarray(f[:, NM + NF:2 * NM]).reshape(N, C, H, W),
    )


# revision 6
# speedup vs baseline: 56.3198x; 56.3198x over previous
"""Trainium2 Bass kernel for nn_ConeGeometryNet.

Math: for each point p=(px,py) and each 2x2 cone matrix `dirs` (rows are the
two edge directions), the reference computes
  - containment: lambda = solve(dirs^T, p) >= 0 (both coords)
  - distance:    min over the two edge lines of perpendicular distance |cross(unit_edge, p)|

Both reduce to two linear functionals of p per matrix.  With
  s0 = d*px - c*py  (= lambda0 * det),   s1 = a*py - b*px  (= lambda1 * det),
  n0 = |(a,b)|, n1 = |(c,d)|, sgn = sign(det):
  u = sgn*s1/n0,  v = sgn*s0/n1
then
  inside = (lambda0>=0 & lambda1>=0) = (min(u,v) >= 0)
  dist   = min(|u|, |v|)
  inner  = inside*dist = relu(min(u,v))      (when inside, dist == min(u,v))
  outer  = dist - inner
So the per-point work is a (2 -> 2*500) matmul with host-precomputed
coefficients plus a handful of elementwise ops, and the kernel is dominated
by ~324 MB of output DMA writes (memory-bound, as expected).

Sharding: data-parallel over points, 8 cores x 6250 points; frame/window
coefficients replicated.  Outputs concatenated on axis 0 on the host.
"""

import os
import sys

import numpy as np

for _p in ("/opt/trn_rl_repo",):
    if os.path.isdir(_p) and _p not in sys.path:
        sys.path.insert(0, _p)

N, C, H, W, D = 50000, 20, 5, 4, 2
NCORES = 8
NS = N // NCORES          # 6250 points per core
TP = 125                  # points per tile (partition dim)
NT = NS // TP             # 50 tiles per core
NF = C * H                # 100 frame cones
NW = C * H * W            # 400 window cones
NM = NF + NW              # 500 cones total
I32_COLS = NM + NF + C    # inside(500) | house(100) | containment(20)
F32_COLS = 2 * NM         # inner(500) | outer(500)

_RUNNER = None


def _coeffs(frame, windows):
    """(2, 2*NM) f32 coefficient matrix: cols [u_0..u_499 | v_0..v_499]."""
    mats = np.concatenate(
        [np.asarray(frame, np.float64).reshape(-1, 2, 2),
         np.asarray(windows, np.float64).reshape(-1, 2, 2)], axis=0)  # (NM,2,2)
    a, b = mats[:, 0, 0], mats[:, 0, 1]
    c_, d = mats[:, 1, 0], mats[:, 1, 1]
    det = a * d - b * c_
    sgn = np.where(det >= 0.0, 1.0, -1.0)
    n0 = np.hypot(a, b)
    n1 = np.hypot(c_, d)
    K = np.empty((2, 2 * NM), np.float64)
    K[0, :NM] = -sgn * b / n0      # u = sgn*(a*py - b*px)/n0
    K[1, :NM] = sgn * a / n0
    K[0, NM:] = sgn * d / n1       # v = sgn*(d*px - c*py)/n1
    K[1, NM:] = -sgn * c_ / n1
    return np.ascontiguousarray(K.astype(np.float32))


def _build_nc(repeat=1):
    """Build the per-core program.  `repeat` > 1 re-runs the tile loop that
    many times (same outputs) — used only for steady-state timing, where
    (t(r1) - t(r0)) / (r1 - r0) cancels dispatch/transfer overheads."""
    import concourse.bacc as bacc
    import concourse.mybir as mybir
    from concourse import tile

    dt = mybir.dt
    Alu = mybir.AluOpType
    Act = mybir.ActivationFunctionType
    Axis = mybir.AxisListType

    nc = bacc.Bacc(None, target_bir_lowering=False)
    pts = nc.declare_dram_parameter("points", [NS, D], dt.float32, isOutput=False)
    cof = nc.declare_dram_parameter("coeffs", [D, 2 * NM], dt.float32, isOutput=False)
    o_f32 = nc.declare_dram_parameter("out_f32", [NS, F32_COLS], dt.float32, isOutput=True)
    o_i32 = nc.declare_dram_parameter("out_i32", [NS, I32_COLS], dt.int32, isOutput=True)

    with tile.TileContext(nc) as tc:
        with (
            tc.tile_pool(name="const", bufs=1) as cpool,
            tc.tile_pool(name="work", bufs=3) as wpool,
            tc.tile_pool(name="pu", bufs=3, space="PSUM") as pu,
            tc.tile_pool(name="pv", bufs=3, space="PSUM") as pv,
        ):
            # Transposed points (2, NS) so each tile's lhsT is a free-dim slice.
            PT = cpool.tile([D, NS], dt.float32)
            CHUNK = 1250
            for c0 in range(0, NS, CHUNK):
                nc.sync.dma_start(
                    out=PT[:, c0:c0 + CHUNK],
                    in_=pts[c0:c0 + CHUNK, :].rearrange("n d -> d n"),
                )
            K = cpool.tile([D, 2 * NM], dt.float32)
            nc.sync.dma_start(out=K[:], in_=cof[:, :])

            for t in range(NT * repeat):
                t = t % NT
                lhsT = PT[:, t * TP:(t + 1) * TP]
                U = pu.tile([TP, NM], dt.float32)
                V = pv.tile([TP, NM], dt.float32)
                nc.tensor.matmul(U[:], lhsT, K[:, 0:NM], start=True, stop=True)
                nc.tensor.matmul(V[:], lhsT, K[:, NM:2 * NM], start=True, stop=True)

                mn = wpool.tile([TP, NM], dt.float32)
                vs = wpool.tile([TP, NM], dt.float32)
                au = wpool.tile([TP, NM], dt.float32)
                av = wpool.tile([TP, NM], dt.float32)
                dd = wpool.tile([TP, NM], dt.float32)
                fout = wpool.tile([TP, F32_COLS], dt.float32)
                iout = wpool.tile([TP, I32_COLS], dt.int32)
                maxw = wpool.tile([TP, NF], dt.int32)

                # tensor_tensor may read at most one PSUM operand -> stage V in SBUF
                nc.vector.tensor_copy(vs[:], V[:])
                nc.vector.tensor_tensor(mn[:], U[:], vs[:], op=Alu.min)
                nc.scalar.activation(au[:], U[:], Act.Abs)
                nc.scalar.activation(av[:], vs[:], Act.Abs)
                # inner = relu(min(u,v))
                nc.scalar.activation(fout[:, 0:NM], mn[:], Act.Relu)
                # dist = min(|u|, |v|);  outer = dist - inner  (Pool: no min, DVE does it)
                nc.vector.tensor_tensor(dd[:], au[:], av[:], op=Alu.min)
                nc.gpsimd.tensor_tensor(fout[:, NM:2 * NM], dd[:], fout[:, 0:NM], op=Alu.subtract)
                # inside = (min(u,v) >= 0) as int32
                nc.gpsimd.tensor_scalar(iout[:, 0:NM], mn[:], 0.0, None, op0=Alu.is_ge)
                # house = insideF & !max_w(insideW)  ==  insideF > max_w(insideW)
                nc.vector.tensor_reduce(
                    maxw[:], iout[:, NF:NM].rearrange("p (g w) -> p g w", w=W),
                    axis=Axis.X, op=Alu.max)
                nc.vector.tensor_tensor(iout[:, NM:NM + NF], iout[:, 0:NF], maxw[:], op=Alu.is_gt)
                # containment = max_h(house)
                nc.vector.tensor_reduce(
                    iout[:, NM + NF:I32_COLS],
                    iout[:, NM:NM + NF].rearrange("p (c h) -> p c h", h=H),
                    axis=Axis.X, op=Alu.max)

                nc.sync.dma_start(out=o_f32[t * TP:(t + 1) * TP, :], in_=fout[:])
                nc.sync.dma_start(out=o_i32[t * TP:(t + 1) * TP, :], in_=iout[:])

    nc.finalize()
    return nc


class _Runner:
    def __init__(self):
        self.nc = _build_nc()

    def run(self, pts, Kc):
        from concourse.bass_utils import run_bass_kernel_spmd

        in_maps = [
            {"points": pts[i * NS:(i + 1) * NS], "coeffs": Kc}
            for i in range(NCORES)
        ]
        return run_bass_kernel_spmd(self.nc, in_maps, list(range(NCORES))).results


def _get_runner():
    global _RUNNER
    if _RUNNER is None:
        _RUNNER = _Runner()
    return _RUNNER


def kernel(points, frame, windows):
    pts = np.ascontiguousarray(np.asarray(points, dtype=np.float32))
    assert pts.shape == (N, D), pts.shape
    Kc = _coeffs(frame, windows)

    rr = _get_runner().run(pts, Kc)
    f = np.concatenate([r["out_f32"] for r in rr], axis=0)
    ii = np.concatenate([r["out_i32"] for r in rr], axis=0)

    return dict(
        embeddings=pts,
        crisp_frame_containment=np.ascontiguousarray(ii[:, 0:NF]).reshape(N, C, H),
        crisp_window_containment=np.ascontiguousarray(ii[:, NF:NM]).reshape(N, C, H, W),
        crisp_house_containment=np.ascontiguousarray(ii[:, NM:NM + NF]).reshape(N, C, H),
        crisp_containment=np.ascontiguousarray(ii[:, NM + NF:I32_COLS]),
        inner_frame_distance=np.ascontiguousarray(f[:, 0:NF]).reshape(N, C, H),
        outer_frame_distance=np.ascontiguousarray(f[:, NM:NM + NF]).reshape(N, C, H),
        inner_window_distances=np.ascontiguousarray(f[:, NF:NM]).reshape(N, C, H, W),
        outer_window_distances=np.ascontiguousarray(f[:, NM + NF:2 * NM]).reshape(N, C, H, W),
    )


# revision 7
# speedup vs baseline: 149.0483x; 2.6465x over previous
"""Trainium2 Bass kernel for nn_ConeGeometryNet.

Math: for each point p=(px,py) and each 2x2 cone matrix `dirs` (rows are the
two edge directions), the reference computes
  - containment: lambda = solve(dirs^T, p) >= 0 (both coords)
  - distance:    min over the two edge lines of perpendicular distance |cross(unit_edge, p)|

Both reduce to two linear functionals of p per matrix.  With
  s0 = d*px - c*py  (= lambda0 * det),   s1 = a*py - b*px  (= lambda1 * det),
  n0 = |(a,b)|, n1 = |(c,d)|, sgn = sign(det):
  u = sgn*s1/n0,  v = sgn*s0/n1
then
  inside = (lambda0>=0 & lambda1>=0) = (min(u,v) >= 0)
  dist   = min(|u|, |v|)
  inner  = inside*dist = relu(min(u,v))      (when inside, dist == min(u,v))
  outer  = dist - inner
So the per-point work is a (2 -> 2*500) matmul with host-precomputed
coefficients plus a handful of elementwise ops, and the kernel is dominated
by ~324 MB of output DMA writes (memory-bound, as expected).

Sharding: data-parallel over points, 8 cores x 6250 points; frame/window
coefficients replicated.  Outputs concatenated on axis 0 on the host.
"""

import os
import sys

import numpy as np

for _p in ("/opt/trn_rl_repo",):
    if os.path.isdir(_p) and _p not in sys.path:
        sys.path.insert(0, _p)

N, C, H, W, D = 50000, 20, 5, 4, 2
NCORES = 8
NS = N // NCORES          # 6250 points per core
TP = 125                  # points per tile (partition dim)
NT = NS // TP             # 50 tiles per core
NF = C * H                # 100 frame cones
NW = C * H * W            # 400 window cones
NM = NF + NW              # 500 cones total
I32_COLS = NM + NF + C    # inside(500) | house(100) | containment(20)
F32_COLS = 2 * NM         # inner(500) | outer(500)

_RUNNER = None


def _coeffs(frame, windows):
    """(2, 2*NM) f32 coefficient matrix: cols [u_0..u_499 | v_0..v_499]."""
    mats = np.concatenate(
        [np.asarray(frame, np.float64).reshape(-1, 2, 2),
         np.asarray(windows, np.float64).reshape(-1, 2, 2)], axis=0)  # (NM,2,2)
    a, b = mats[:, 0, 0], mats[:, 0, 1]
    c_, d = mats[:, 1, 0], mats[:, 1, 1]
    det = a * d - b * c_
    sgn = np.where(det >= 0.0, 1.0, -1.0)
    n0 = np.hypot(a, b)
    n1 = np.hypot(c_, d)
    K = np.empty((2, 2 * NM), np.float64)
    K[0, :NM] = -sgn * b / n0      # u = sgn*(a*py - b*px)/n0
    K[1, :NM] = sgn * a / n0
    K[0, NM:] = sgn * d / n1       # v = sgn*(d*px - c*py)/n1
    K[1, NM:] = -sgn * c_ / n1
    return np.ascontiguousarray(K.astype(np.float32))


def _build_nc(repeat=1):
    """Build the per-core program.  `repeat` > 1 re-runs the tile loop that
    many times (same outputs) — used only for steady-state timing, where
    (t(r1) - t(r0)) / (r1 - r0) cancels dispatch/transfer overheads."""
    import concourse.bacc as bacc
    import concourse.mybir as mybir
    from concourse import tile

    dt = mybir.dt
    Alu = mybir.AluOpType
    Act = mybir.ActivationFunctionType
    Axis = mybir.AxisListType

    nc = bacc.Bacc(None, target_bir_lowering=False)
    pts = nc.declare_dram_parameter("points", [NS, D], dt.float32, isOutput=False)
    cof = nc.declare_dram_parameter("coeffs", [D, 2 * NM], dt.float32, isOutput=False)
    o_f32 = nc.declare_dram_parameter("out_f32", [NS, F32_COLS], dt.float32, isOutput=True)
    o_i32 = nc.declare_dram_parameter("out_i32", [NS, I32_COLS], dt.int32, isOutput=True)

    with tile.TileContext(nc) as tc:
        with (
            tc.tile_pool(name="const", bufs=1) as cpool,
            tc.tile_pool(name="work", bufs=3) as wpool,
            tc.tile_pool(name="pu", bufs=3, space="PSUM") as pu,
            tc.tile_pool(name="pv", bufs=3, space="PSUM") as pv,
        ):
            # Transposed points (2, NS) so each tile's lhsT is a free-dim slice.
            PT = cpool.tile([D, NS], dt.float32)
            CHUNK = 1250
            for c0 in range(0, NS, CHUNK):
                nc.sync.dma_start(
                    out=PT[:, c0:c0 + CHUNK],
                    in_=pts[c0:c0 + CHUNK, :].rearrange("n d -> d n"),
                )
            K = cpool.tile([D, 2 * NM], dt.float32)
            nc.sync.dma_start(out=K[:], in_=cof[:, :])

            for t in range(NT * repeat):
                t = t % NT
                lhsT = PT[:, t * TP:(t + 1) * TP]
                U = pu.tile([TP, NM], dt.float32)
                V = pv.tile([TP, NM], dt.float32)
                nc.tensor.matmul(U[:], lhsT, K[:, 0:NM], start=True, stop=True)
                nc.tensor.matmul(V[:], lhsT, K[:, NM:2 * NM], start=True, stop=True)

                mn = wpool.tile([TP, NM], dt.float32)
                vs = wpool.tile([TP, NM], dt.float32)
                au = wpool.tile([TP, NM], dt.float32)
                av = wpool.tile([TP, NM], dt.float32)
                dd = wpool.tile([TP, NM], dt.float32)
                fout = wpool.tile([TP, F32_COLS], dt.float32)
                iout = wpool.tile([TP, I32_COLS], dt.int32)
                maxw = wpool.tile([TP, NF], dt.int32)

                # tensor_tensor may read at most one PSUM operand -> stage V in SBUF
                nc.vector.tensor_copy(vs[:], V[:])
                nc.vector.tensor_tensor(mn[:], U[:], vs[:], op=Alu.min)
                nc.scalar.activation(au[:], U[:], Act.Abs)
                nc.scalar.activation(av[:], vs[:], Act.Abs)
                # inner = relu(min(u,v))
                nc.scalar.activation(fout[:, 0:NM], mn[:], Act.Relu)
                # dist = min(|u|, |v|);  outer = dist - inner
                # (GpSimd measured ~5x slower than DVE for these; keep off Pool)
                nc.vector.tensor_tensor(dd[:], au[:], av[:], op=Alu.min)
                nc.vector.tensor_tensor(fout[:, NM:2 * NM], dd[:], fout[:, 0:NM], op=Alu.subtract)
                # inside = (min(u,v) >= 0) as int32
                nc.vector.tensor_scalar(iout[:, 0:NM], mn[:], 0.0, None, op0=Alu.is_ge)
                # house = insideF & !max_w(insideW)  ==  insideF > max_w(insideW)
                nc.vector.tensor_reduce(
                    maxw[:], iout[:, NF:NM].rearrange("p (g w) -> p g w", w=W),
                    axis=Axis.X, op=Alu.max)
                nc.vector.tensor_tensor(iout[:, NM:NM + NF], iout[:, 0:NF], maxw[:], op=Alu.is_gt)
                # containment = max_h(house)
                nc.vector.tensor_reduce(
                    iout[:, NM + NF:I32_COLS],
                    iout[:, NM:NM + NF].rearrange("p (c h) -> p c h", h=H),
                    axis=Axis.X, op=Alu.max)

                nc.sync.dma_start(out=o_f32[t * TP:(t + 1) * TP, :], in_=fout[:])
                nc.sync.dma_start(out=o_i32[t * TP:(t + 1) * TP, :], in_=iout[:])

    nc.finalize()
    return nc


class _Runner:
    def __init__(self):
        self.nc = _build_nc()

    def run(self, pts, Kc):
        from concourse.bass_utils import run_bass_kernel_spmd

        in_maps = [
            {"points": pts[i * NS:(i + 1) * NS], "coeffs": Kc}
            for i in range(NCORES)
        ]
        return run_bass_kernel_spmd(self.nc, in_maps, list(range(NCORES))).results


def _get_runner():
    global _RUNNER
    if _RUNNER is None:
        _RUNNER = _Runner()
    return _RUNNER


def kernel(points, frame, windows):
    pts = np.ascontiguousarray(np.asarray(points, dtype=np.float32))
    assert pts.shape == (N, D), pts.shape
    Kc = _coeffs(frame, windows)

    rr = _get_runner().run(pts, Kc)
    f = np.concatenate([r["out_f32"] for r in rr], axis=0)
    ii = np.concatenate([r["out_i32"] for r in rr], axis=0)

    return dict(
        embeddings=pts,
        crisp_frame_containment=np.ascontiguousarray(ii[:, 0:NF]).reshape(N, C, H),
        crisp_window_containment=np.ascontiguousarray(ii[:, NF:NM]).reshape(N, C, H, W),
        crisp_house_containment=np.ascontiguousarray(ii[:, NM:NM + NF]).reshape(N, C, H),
        crisp_containment=np.ascontiguousarray(ii[:, NM + NF:I32_COLS]),
        inner_frame_distance=np.ascontiguousarray(f[:, 0:NF]).reshape(N, C, H),
        outer_frame_distance=np.ascontiguousarray(f[:, NM:NM + NF]).reshape(N, C, H),
        inner_window_distances=np.ascontiguousarray(f[:, NF:NM]).reshape(N, C, H, W),
        outer_window_distances=np.ascontiguousarray(f[:, NM + NF:2 * NM]).reshape(N, C, H, W),
    )


# revision 11
# speedup vs baseline: 335.2322x; 2.2492x over previous
"""Trainium2 Bass kernel for nn_ConeGeometryNet.

Math: for each point p=(px,py) and each 2x2 cone matrix `dirs` (rows are the
two edge directions), the reference computes
  - containment: lambda = solve(dirs^T, p) >= 0 (both coords)
  - distance:    min over the two edge lines of perpendicular distance |cross(unit_edge, p)|

Both reduce to two linear functionals of p per matrix.  With
  s0 = d*px - c*py  (= lambda0 * det),   s1 = a*py - b*px  (= lambda1 * det),
  n0 = |(a,b)|, n1 = |(c,d)|, sgn = sign(det):
  u = sgn*s1/n0,  v = sgn*s0/n1
then
  inside = (lambda0>=0 & lambda1>=0) = (min(u,v) >= 0)
  dist   = min(|u|, |v|)
  inner  = inside*dist = relu(min(u,v))      (when inside, dist == min(u,v))
  outer  = dist - inner
So the per-point work is a (2 -> 2*500) matmul with host-precomputed
coefficients plus a handful of elementwise ops, and the kernel is dominated
by ~324 MB of output DMA writes (memory-bound, as expected).

Sharding: data-parallel over points, 8 cores x 6250 points; frame/window
coefficients replicated.  Outputs concatenated on axis 0 on the host.
"""

import os
import sys

import numpy as np

for _p in ("/opt/trn_rl_repo",):
    if os.path.isdir(_p) and _p not in sys.path:
        sys.path.insert(0, _p)



N, C, H, W, D = 50000, 20, 5, 4, 2
NCORES = 8
NS = N // NCORES          # 6250 points per core
TP = 125                  # points per tile (partition dim)
NT = NS // TP             # 50 tiles per core
NF = C * H                # 100 frame cones
NW = C * H * W            # 400 window cones
NM = NF + NW              # 500 cones total
I32_COLS = NM + NF + C    # inside(500) | house(100) | containment(20)
F32_COLS = 2 * NM         # inner(500) | outer(500)

_RUNNER = None


def _coeffs(frame, windows):
    """(2, 2*NM) f32 coefficient matrix: cols [u_0..u_499 | v_0..v_499]."""
    mats = np.concatenate(
        [np.asarray(frame, np.float64).reshape(-1, 2, 2),
         np.asarray(windows, np.float64).reshape(-1, 2, 2)], axis=0)  # (NM,2,2)
    a, b = mats[:, 0, 0], mats[:, 0, 1]
    c_, d = mats[:, 1, 0], mats[:, 1, 1]
    det = a * d - b * c_
    sgn = np.where(det >= 0.0, 1.0, -1.0)
    n0 = np.hypot(a, b)
    n1 = np.hypot(c_, d)
    K = np.empty((2, 2 * NM), np.float64)
    K[0, :NM] = -sgn * b / n0      # u = sgn*(a*py - b*px)/n0
    K[1, :NM] = sgn * a / n0
    K[0, NM:] = sgn * d / n1       # v = sgn*(d*px - c*py)/n1
    K[1, NM:] = -sgn * c_ / n1
    return np.ascontiguousarray(K.astype(np.float32))


def _build_nc(repeat=1):
    """Build the per-core program.  `repeat` > 1 re-runs the tile loop that
    many times (same outputs) — used only for steady-state timing, where
    (t(r1) - t(r0)) / (r1 - r0) cancels dispatch/transfer overheads."""
    import concourse.bacc as bacc
    import concourse.mybir as mybir
    from concourse import tile

    dt = mybir.dt
    Alu = mybir.AluOpType
    Act = mybir.ActivationFunctionType
    Axis = mybir.AxisListType

    nc = bacc.Bacc(None, target_bir_lowering=False)
    pts = nc.declare_dram_parameter("points", [NS, D], dt.float32, isOutput=False)
    cof = nc.declare_dram_parameter("coeffs", [D, 2 * NM], dt.float32, isOutput=False)
    o_f32 = nc.declare_dram_parameter("out_f32", [NS, F32_COLS], dt.float32, isOutput=True)
    o_i32 = nc.declare_dram_parameter("out_i32", [NS, I32_COLS], dt.int32, isOutput=True)

    with tile.TileContext(nc) as tc:
        with (
            tc.tile_pool(name="const", bufs=1) as cpool,
            tc.tile_pool(name="work", bufs=3) as wpool,
            tc.tile_pool(name="pu", bufs=3, space="PSUM") as pu,
            tc.tile_pool(name="pv", bufs=3, space="PSUM") as pv,
        ):
            # Transposed points (2, NS) so each tile's lhsT is a free-dim slice.
            PT = cpool.tile([D, NS], dt.float32)
            CHUNK = 1250
            for c0 in range(0, NS, CHUNK):
                nc.sync.dma_start(
                    out=PT[:, c0:c0 + CHUNK],
                    in_=pts[c0:c0 + CHUNK, :].rearrange("n d -> d n"),
                )
            K = cpool.tile([D, 2 * NM], dt.float32)
            nc.sync.dma_start(out=K[:], in_=cof[:, :])

            for t in range(NT * repeat):
                t = t % NT
                lhsT = PT[:, t * TP:(t + 1) * TP]
                U = pu.tile([TP, NM], dt.float32)
                V = pv.tile([TP, NM], dt.float32)
                nc.tensor.matmul(U[:], lhsT, K[:, 0:NM], start=True, stop=True)
                nc.tensor.matmul(V[:], lhsT, K[:, NM:2 * NM], start=True, stop=True)

                mn = wpool.tile([TP, NM], dt.float32)
                vs = wpool.tile([TP, NM], dt.float32)
                au = wpool.tile([TP, NM], dt.float32)
                av = wpool.tile([TP, NM], dt.float32)
                dd = wpool.tile([TP, NM], dt.float32)
                fout = wpool.tile([TP, F32_COLS], dt.float32)
                iout = wpool.tile([TP, I32_COLS], dt.int32)
                maxw = wpool.tile([TP, NF], dt.int32)

                # tensor_tensor may read at most one PSUM operand -> stage V in SBUF
                nc.vector.tensor_copy(vs[:], V[:])
                nc.vector.tensor_tensor(mn[:], U[:], vs[:], op=Alu.min)
                nc.scalar.activation(au[:], U[:], Act.Abs)
                nc.scalar.activation(av[:], vs[:], Act.Abs)
                # inner = relu(min(u,v))
                nc.scalar.activation(fout[:, 0:NM], mn[:], Act.Relu)
                # dist = min(|u|, |v|);  outer = dist - inner
                # (GpSimd measured ~5x slower than DVE for these; keep off Pool)
                nc.vector.tensor_tensor(dd[:], au[:], av[:], op=Alu.min)
                nc.vector.tensor_tensor(fout[:, NM:2 * NM], dd[:], fout[:, 0:NM], op=Alu.subtract)
                # inside = (min(u,v) >= 0) as int32
                nc.vector.tensor_scalar(iout[:, 0:NM], mn[:], 0.0, None, op0=Alu.is_ge)
                # house = insideF & !max_w(insideW)  ==  insideF > max_w(insideW)
                nc.vector.tensor_reduce(
                    maxw[:], iout[:, NF:NM].rearrange("p (g w) -> p g w", w=W),
                    axis=Axis.X, op=Alu.max)
                nc.vector.tensor_tensor(iout[:, NM:NM + NF], iout[:, 0:NF], maxw[:], op=Alu.is_gt)
                # containment = max_h(house)
                nc.vector.tensor_reduce(
                    iout[:, NM + NF:I32_COLS],
                    iout[:, NM:NM + NF].rearrange("p (c h) -> p c h", h=H),
                    axis=Axis.X, op=Alu.max)

                nc.sync.dma_start(out=o_f32[t * TP:(t + 1) * TP, :], in_=fout[:])
                nc.sync.dma_start(out=o_i32[t * TP:(t + 1) * TP, :], in_=iout[:])

    nc.finalize()
    return nc


class _Runner:
    def __init__(self):
        self.nc = _build_nc()

    def run(self, pts, Kc):
        from concourse.bass_utils import run_bass_kernel_spmd

        in_maps = [
            {"points": pts[i * NS:(i + 1) * NS], "coeffs": Kc}
            for i in range(NCORES)
        ]
        return run_bass_kernel_spmd(self.nc, in_maps, list(range(NCORES))).results


def _get_runner():
    global _RUNNER
    if _RUNNER is None:
        _RUNNER = _Runner()
    return _RUNNER


def kernel(points, frame, windows):
    pts = np.ascontiguousarray(np.asarray(points, dtype=np.float32))
    assert pts.shape == (N, D), pts.shape
    Kc = _coeffs(frame, windows)

    rr = _get_runner().run(pts, Kc)
    f = np.concatenate([r["out_f32"] for r in rr], axis=0)
    ii = np.concatenate([r["out_i32"] for r in rr], axis=0)

    return dict(
        embeddings=pts,
        crisp_frame_containment=np.ascontiguousarray(ii[:, 0:NF]).reshape(N, C, H),
        crisp_window_containment=np.ascontiguousarray(ii[:, NF:NM]).reshape(N, C, H, W),
        crisp_house_containment=np.ascontiguousarray(ii[:, NM:NM + NF]).reshape(N, C, H),
        crisp_containment=np.ascontiguousarray(ii[:, NM + NF:I32_COLS]),
        inner_frame_distance=np.ascontiguousarray(f[:, 0:NF]).reshape(N, C, H),
        outer_frame_distance=np.ascontiguousarray(f[:, NM:NM + NF]).reshape(N, C, H),
        inner_window_distances=np.ascontiguousarray(f[:, NF:NM]).reshape(N, C, H, W),
        outer_window_distances=np.ascontiguousarray(f[:, NM + NF:2 * NM]).reshape(N, C, H, W),
    )
